# revision 1
# baseline (speedup 1.0000x reference)
"""BigBird sparse attention kernel for Trainium2 (8 NeuronCores).

Problem (hardcoded): B=2, S=2048, H=16, D=64, block=128, G=128 global
tokens, R=64 random tokens, attn_mask is all-zeros by construction
(spec fill="zeros").

Math notes (mask == 0):
  * Diagonal branch: standard per-(b, block, head) softmax attention
    within each 128-token diagonal block.
  * Global branch: the reference contracts softmax weights only over
    their own row (einsum 'bhgs,bghd->bghd'), so the contribution is
    v[:, :G] * rowsum(softmax) == v[:, :G] (rowsum == 1 up to fp
    rounding).
  * Random branch: same structure ('bhnm,bmhd->bnhd' with size-1
    broadcast), contribution is v[:, r] * rowsum(softmax) == v[:, r],
    scatter-added per occurrence of each random index.
  Both reduce to out[:, s] += cnt2[s] * v[:, s] with
  cnt2 = bincount(rand_indices) + (s < G).

Sharding: each of the 8 cores gets one (batch, 4-head group):
core c -> b = c // 4, heads 4*(c%4) .. 4*(c%4)+4. All branches are
independent per (b, h); no collectives.

Numerics / layout:
  * q, k are pre-transposed on host to (d, s) per head and split into
    bf16 hi/lo pairs. QK^T is computed as Kh'Qh + Kh'Ql + Kl'Qh (three
    accumulating 64-contraction matmuls, ~2^-16 relative score error;
    the dropped Kl'Ql term is ~2^-32).
  * softmax denominator comes for free from an appended ones column on
    V; max-subtraction is skipped (scores are bounded by ~|6| for this
    problem scale so exp cannot overflow).
  * exp() outputs f16 weights; V is host-split into f16 hi/lo planes,
    so PV is two accumulating f16 matmuls (W-f16 rounding dominates the
    total error at ~3e-5 scale-relative absmax).
  * All 4 heads of a block share one PSUM score bank (128x512) so the
    exp is a single ACT op (the 352-cycle ACT fixed overhead would
    otherwise dominate), and one PSUM output bank (128x4x65).
"""

import numpy as np

B, S, H, D = 2, 2048, 16, 64
BS = 128          # block size
NB = S // BS      # 16 diagonal blocks
G = 128           # num global tokens
SCALE = 1.0 / float(D) ** 0.5
NCORES = 8
HPC = 4           # heads per core
QCH = [2, 2, 4, 4, 4]   # s-blocks per qk DMA chunk (small first for fast start)
QOF = [0, 2, 4, 8, 12]  # chunk offsets (blocks)
VCH = [1, 1] + [2] * 7  # v chunks: singles first so PV starts early
VOF = [0, 1, 2, 4, 6, 8, 10, 12, 14]
OCH = [2] * 7 + [1, 1]  # out chunks: singles last for a short tail
OOF = [0, 2, 4, 6, 8, 10, 12, 14, 15]

_cached = {}


def _build_program():
    import concourse.bass as bass
    import concourse.tile as tile
    from concourse import bacc, mybir

    f32 = mybir.dt.float32
    f16 = mybir.dt.float16
    bf16 = mybir.dt.bfloat16
    AF = mybir.ActivationFunctionType
    ALU = mybir.AluOpType

    nc = bacc.Bacc(
        "TRN2",
        target_bir_lowering=False,
        debug=False,
        enable_asserts=False,
        num_devices=NCORES,
    )
    # qk[pair][chunk][row][qk][sq]: rows of a pair hold heads (2i, 2i+1)
    # as partition p = (h % 2) * 64 + d; q/k stored as f16 (the PE is
    # clock-limited, so QK^T uses a single f16 matmul per head);
    # pre-tiled on host so each chunk load is one fully contiguous
    # block. First chunks are small so the first matmul starts early,
    # later ones are large for big DMA descriptors.
    qk = nc.dram_tensor("qk", [2, 128 * 2 * S], f16, kind="ExternalInput").ap()
    # v chunks of [row][blk][hl][h][d+1]: f16 hi/lo planes; hi plane
    # has ones in col D (softmax denominator), lo plane zeros there.
    # Chunk-tiled (VCH) for large per-partition descriptor runs.
    VROW = 2 * HPC * (D + 1)
    v = nc.dram_tensor("v", [128 * 2 * HPC * (D + 1) * NB], f16,
                       kind="ExternalInput").ap()
    # unnormalized PV plus rowsum column, chunk-tiled (OCH);
    # normalization happens on host.
    OROW = HPC * (D + 1)
    out = nc.dram_tensor("out", [128 * HPC * (D + 1) * NB], f32,
                         kind="ExternalOutput").ap()

    OD = D + 2  # 8-byte-aligned per-head stride in the output PSUM bank

    with tile.TileContext(nc) as tc:
        with (
            tc.tile_pool(name="qk", bufs=8) as qkpool,
            tc.tile_pool(name="vp", bufs=9) as vpool,
            tc.tile_pool(name="wp", bufs=6) as wpool,
            tc.tile_pool(name="outp", bufs=5) as opool,
            tc.tile_pool(name="stps", bufs=2, space="PSUM") as stpool,
            tc.tile_pool(name="ops", bufs=2, space="PSUM") as oppool,
        ):
            state = [None] * NB  # per-block tiles

            # prefetch all q/k chunks up-front so the DMA queues are
            # saturated from t=0
            qk_tiles = []  # per block: (tile_pair0, tile_pair1, in-chunk idx)
            for off, sz in zip(QOF, QCH):
                ln = sz * BS
                base = 128 * 2 * off * BS
                cnt = 128 * 2 * ln
                pts = []
                for pair in range(2):
                    t = qkpool.tile([128, 2, ln], f16, tag=f"qk{pair}")
                    nc.sync.dma_start(
                        t[:],
                        qk[pair, base : base + cnt].rearrange(
                            "(p a s) -> p a s", p=128, a=2
                        ),
                    )
                    pts.append(t)
                for i in range(sz):
                    qk_tiles.append((pts[0], pts[1], i))

            vmap = {}  # block -> (chunk off, size, in-chunk idx)
            for off, sz in zip(VOF, VCH):
                for i in range(sz):
                    vmap[off + i] = (off, sz, i)
            v_chunk = {}  # chunk off -> tile

            def issue_v(ci):
                off, sz = VOF[ci], VCH[ci]
                v_t = vpool.tile([128, sz, 2, HPC, D + 1], f16, tag="v")
                base = 128 * VROW * off
                nc.scalar.dma_start(
                    v_t[:],
                    v[base : base + 128 * VROW * sz].rearrange(
                        "(p c a h d) -> p c a h d",
                        p=128, c=sz, a=2, h=HPC,
                    ),
                )
                v_chunk[off] = v_t

            # prefetch the first few v chunks; issue the rest 4 blocks
            # ahead of use so the ACT-queue exp waits don't gate the ring
            for ci in range(3):
                issue_v(ci)
            vnext = [3]

            def stage_front(sb):
                """loads + QK^T + exp for block sb"""
                t0, t1, sbl = qk_tiles[sb]
                ssl = slice(sbl * BS, (sbl + 1) * BS)
                state[sb] = {"qk": (t0, t1)}

                while vnext[0] < len(VCH) and VOF[vnext[0]] <= sb + 4:
                    issue_v(vnext[0])
                    vnext[0] += 1
                off, sz, vi = vmap[sb]
                state[sb]["v_t"] = v_chunk[off]
                state[sb]["vbl"] = vi

                # One PSUM bank per PE row-group: concurrent row-tiled
                # matmuls to the same bank are a hardware fault, so heads
                # with sub=0 (PE rows 0-63) share bank "e" and sub=1 heads
                # (rows 64-127) share bank "o". Within a bank all matmuls
                # use the same row group and serialize naturally.
                st_e = stpool.tile([128, 2 * BS], f32, tag="st_e")
                st_o = stpool.tile([128, 2 * BS], f32, tag="st_o")
                for h in range(HPC):
                    pair, sub = divmod(h, 2)
                    st = (st_e, st_o)[sub]
                    qkt = state[sb]["qk"][pair]
                    dsl = slice(sub * 64, (sub + 1) * 64)
                    col = slice(pair * BS, (pair + 1) * BS)
                    # S^T[k,q] = K'Q (f16)
                    nc.tensor.matmul(
                        st[:, col], lhsT=qkt[dsl, 1, ssl], rhs=qkt[dsl, 0, ssl],
                        start=True, stop=True,
                    )
                w_e = wpool.tile([128, 2 * BS], f16, tag="w_e")
                nc.scalar.activation(w_e[:], st_e[:], AF.Exp, scale=SCALE)
                w_o = wpool.tile([128, 2 * BS], f16, tag="w_o")
                nc.scalar.activation(w_o[:], st_o[:], AF.Exp, scale=SCALE)
                state[sb].update(w_e=w_e, w_o=w_o)

            omap = {}
            for off, sz in zip(OOF, OCH):
                for i in range(sz):
                    omap[off + i] = (off, sz, i)

            def stage_back(sb):
                """PV + normalize + store for block sb"""
                stt = state[sb]
                v_t = stt["v_t"]
                vbl = stt["vbl"]
                # one 130-col matmul per head: moving = [Vh|1 , Vl|0] planes
                # side by side; hi/lo partial outputs summed during PSUM
                # evacuation. Per-head stride 2*(D+1)+2 keeps 8B alignment.
                HOD = 2 * (D + 1) + 2  # 132
                o_a = oppool.tile([128, 2, HOD], f32, tag="o_a")
                o_b = oppool.tile([128, 2, HOD], f32, tag="o_b")
                for h in range(HPC):
                    pair, sub = divmod(h, 2)
                    w_t = (stt["w_e"], stt["w_o"])[sub]
                    wcol = slice(pair * BS, (pair + 1) * BS)
                    o_ps = (o_a, o_b)[h // 2]
                    nc.tensor.matmul(
                        o_ps[:, h % 2, 0 : 2 * (D + 1)],
                        lhsT=w_t[:, wcol],
                        rhs=v_t[:, vbl, :, h, :],
                        start=True, stop=True,
                    )
                # evacuate o_hi + o_lo (and rowsum) to SBUF via an X-axis
                # reduce over the interleaved plane pairs (a tensor op may
                # read only ONE PSUM operand); host divides and adds the
                # cnt2*v global/random contribution. Output is buffered in
                # OCH-sized chunks for large DMA descriptor runs.
                ooff, osz, oi = omap[sb]
                if oi == 0:
                    out_t = opool.tile([128, osz, HPC, D + 1], f32, tag="out")
                    state[sb]["out_t"] = out_t
                else:
                    out_t = state[sb - oi]["out_t"]
                nc.vector.tensor_reduce(
                    out_t[:, oi, 0:2, :],
                    o_a[:, :, 0 : 2 * (D + 1)].rearrange(
                        "p hh (pl c) -> p hh c pl", pl=2
                    ),
                    mybir.AxisListType.X, ALU.add,
                )
                nc.vector.tensor_reduce(
                    out_t[:, oi, 2:4, :],
                    o_b[:, :, 0 : 2 * (D + 1)].rearrange(
                        "p hh (pl c) -> p hh c pl", pl=2
                    ),
                    mybir.AxisListType.X, ALU.add,
                )
                if oi == osz - 1:
                    base = 128 * OROW * ooff
                    # early out chunks ride the SWDGE ring; the second
                    # half uses the SP HW ring, which is idle once the qk
                    # prefetch drains, pulling the write tail in.
                    dma_eng = nc.sync if ooff >= 8 else nc.gpsimd
                    dma_eng.dma_start(
                        out[base : base + 128 * OROW * osz].rearrange(
                            "(p c h d) -> p c h d", p=128, c=osz, h=HPC
                        ),
                        out_t[:],
                    )

            # 2-block software skew: PE works on block sb's QK^T while
            # ACT runs exp(sb-1) and PE then does PV(sb-2). (Deeper skew
            # measured worse: it piles the last PV stages into the tail.)
            SKEW = 2
            for sb in range(NB + SKEW):
                if sb < NB:
                    stage_front(sb)
                if sb >= SKEW:
                    stage_back(sb - SKEW)
    nc.compile()
    return nc


def _get_nc():
    if "nc" not in _cached:
        _cached["nc"] = _build_program()
    return _cached["nc"]


def _split_hi_lo_bf16(x):
    import ml_dtypes

    hi = x.astype(ml_dtypes.bfloat16)
    lo = (x - hi.astype(np.float32)).astype(ml_dtypes.bfloat16)
    return hi, lo


def _make_in_maps(q, k, v, rand_indices):
    import ml_dtypes

    q = np.asarray(q, dtype=np.float32)
    k = np.asarray(k, dtype=np.float32)
    v = np.asarray(v, dtype=np.float32)

    in_maps = []
    for c in range(NCORES):
        b, hg = divmod(c, 4)
        hsl = slice(HPC * hg, HPC * (hg + 1))
        # (S, HPC, D) -> (HPC, D, S) -> (2 pairs, 128, S)
        qT = np.ascontiguousarray(q[b, :, hsl, :].transpose(1, 2, 0)).reshape(2, 128, S)
        kT = np.ascontiguousarray(k[b, :, hsl, :].transpose(1, 2, 0)).reshape(2, 128, S)
        # full4[qk][pair][row][s] in f16
        full4 = np.stack([qT, kT]).astype(np.float16)
        # flat per pair: chunks of [row][qk][sq], each contiguous
        qkc = np.empty((2, 128 * 2 * S), np.float16)
        for pair in range(2):
            pos = 0
            for off, sz in zip(QOF, QCH):
                ch = full4[:, pair, :, off * BS : (off + sz) * BS]
                ch = np.ascontiguousarray(ch.transpose(1, 0, 2))  # row,qk,sq
                qkc[pair, pos : pos + ch.size] = ch.ravel()
                pos += ch.size

        vc = v[b, :, hsl, :]  # (S, HPC, D) f32
        vh = vc.astype(np.float16)
        vl = (vc - vh.astype(np.float32)).astype(np.float16)
        vhl = np.zeros((S, 2, HPC, D + 1), np.float16)
        vhl[:, 0, :, 0:D] = vh
        vhl[:, 1, :, 0:D] = vl
        vhl[:, 0, :, D] = 1.0  # softmax denominator column
        # chunk-tile: each VCH chunk as [row][blk][hl][h][d], flat
        vhl = vhl.reshape(NB, 128, 2, HPC, D + 1)
        vflat = np.empty(128 * 2 * HPC * (D + 1) * NB, np.float16)
        pos = 0
        for off, sz in zip(VOF, VCH):
            ch = np.ascontiguousarray(
                vhl[off : off + sz].transpose(1, 0, 2, 3, 4)
            )  # row, blk, hl, h, d
            vflat[pos : pos + ch.size] = ch.ravel()
            pos += ch.size
        in_maps.append({"qk": qkc, "v": vflat})
    return in_maps


def _unpack_out(o):
    """OCH-chunk-tiled flat -> (S, HPC, D+1)"""
    res = np.empty((NB, 128, HPC, D + 1), np.float32)
    pos = 0
    for off, sz in zip(OOF, OCH):
        n = 128 * sz * HPC * (D + 1)
        ch = o[pos : pos + n].reshape(128, sz, HPC, D + 1)
        res[off : off + sz] = ch.transpose(1, 0, 2, 3)
        pos += n
    return res.reshape(S, HPC, D + 1)


def _assemble(results, v, rand_indices):
    out = np.empty((B, S, H, D), dtype=np.float32)
    for c in range(NCORES):
        b, hg = divmod(c, 4)
        o = _unpack_out(results[c]["out"])  # (S, HPC, D+1): [o_unnorm | rowsum]
        out[b, :, HPC * hg : HPC * (hg + 1), :] = o[:, :, 0:D] / o[:, :, D : D + 1]
    # global + random contributions: out[:, s] += cnt2[s] * v[:, s]
    ri = np.asarray(rand_indices).astype(np.int64).ravel()
    cnt = np.bincount(ri, minlength=S).astype(np.float32)
    cnt[:G] += 1.0
    nz = np.nonzero(cnt)[0]
    out[:, nz] += cnt[nz, None, None] * np.asarray(v, np.float32)[:, nz]
    return out


def _run(q, k, v, attn_mask, rand_indices, trace=False, trace_kwargs=None):
    from concourse.bass_utils import run_bass_kernel_spmd

    nc = _get_nc()
    in_maps = _make_in_maps(q, k, v, rand_indices)
    res = run_bass_kernel_spmd(
        nc,
        in_maps,
        list(range(NCORES)),
        trace=trace,
        **(trace_kwargs or {}),
    )
    return _assemble(res.results, v, rand_indices), res


def _reference_fallback(q, k, v, attn_mask, rand_indices):
    """Numpy replica of the reference for the (never expected per spec)
    case of a non-zero attn_mask."""
    q = np.asarray(q, np.float32)
    k = np.asarray(k, np.float32)
    v = np.asarray(v, np.float32)
    m = np.asarray(attn_mask, np.float32)
    ri = np.asarray(rand_indices).astype(np.int64).ravel()

    def softmax(x):
        x = x - x.max(axis=-1, keepdims=True)
        e = np.exp(x)
        return e / e.sum(axis=-1, keepdims=True)

    qb = q.reshape(B, NB, BS, H, D)
    kb = k.reshape(B, NB, BS, H, D)
    vb = v.reshape(B, NB, BS, H, D)
    scores = np.einsum("bnqhd,bnkhd->bnhqk", qb, kb) * SCALE
    mb = m.reshape(B, H, NB, BS, NB, BS)
    idx = np.arange(NB)
    diag = mb[:, :, idx, :, idx, :]  # (NB,B,H,BS,BS)
    scores = scores + diag.transpose(1, 0, 2, 3, 4)
    w = softmax(scores)
    out = np.einsum("bnhqk,bnkhd->bnqhd", w, vb).reshape(B, S, H, D)

    gq = q[:, :G]
    gv = v[:, :G]
    gs = np.einsum("bghd,bshd->bhgs", gq, k) * SCALE + m[:, :, :G, :]
    gw = softmax(gs)
    out[:, :G] += gv * gw.sum(axis=-1).transpose(0, 2, 1)[..., None]

    rq = q[:, ri]
    rv = v[:, ri]
    rs = np.einsum("brhd,bshd->bhrs", rq, k) * SCALE + m[:, :, ri, :]
    rw = softmax(rs)
    rowsum = rw.sum(axis=-1).transpose(0, 2, 1)  # (B,R,H)
    contrib = rv * rowsum[..., None]
    np.add.at(out, (slice(None), ri), contrib)
    return out


def kernel(q, k, v, attn_mask, rand_indices):
    am = np.asarray(attn_mask)
    if am.any():
        return _reference_fallback(q, k, v, attn_mask, rand_indices)
    out, _ = _run(q, k, v, attn_mask, rand_indices, trace=False)
    return out



# revision 9
# speedup vs baseline: 1.2021x; 1.2021x over previous
"""BigBird sparse attention kernel for Trainium2 (8 NeuronCores).

Problem (hardcoded): B=2, S=2048, H=16, D=64, block=128, G=128 global
tokens, R=64 random tokens, attn_mask is all-zeros by construction
(spec fill="zeros").

Math notes (mask == 0):
  * Diagonal branch: standard per-(b, block, head) softmax attention
    within each 128-token diagonal block.
  * Global branch: the reference contracts softmax weights only over
    their own row (einsum 'bhgs,bghd->bghd'), so the contribution is
    v[:, :G] * rowsum(softmax) == v[:, :G] (rowsum == 1 up to fp
    rounding).
  * Random branch: same structure ('bhnm,bmhd->bnhd' with size-1
    broadcast), contribution is v[:, r] * rowsum(softmax) == v[:, r],
    scatter-added per occurrence of each random index.
  Both reduce to out[:, s] += cnt2[s] * v[:, s] with
  cnt2 = bincount(rand_indices) + (s < G).

Sharding: each of the 8 cores gets one (batch, 4-head group):
core c -> b = c // 4, heads 4*(c%4) .. 4*(c%4)+4. All branches are
independent per (b, h); no collectives.

Numerics / layout (tolerance is rel 2e-2; fp8 keeps us ~6e-3):
  * q, k are pre-transposed on host to (d, s) per head in fp8-e4m3.
    QK^T is one fp8 matmul per (head, block) -> f32 PSUM scores.
  * exp runs with scale=1/8 and bias=-1.5 (a global shift cancels in
    the final normalize; it keeps fp8/f16 magnitudes comfortable),
    writing fp8 weights. One ACT op covers a full 512-col PSUM bank
    (2 heads x 2 blocks), so only 16 ACT ops total.
  * V is fp8 with an appended ones column (softmax denominator comes
    free from the PV matmul).
  * PV output (unnormalized, + rowsum col) is evacuated PSUM->SBUF as
    f16 (Vector/GpSimd alternate), DMA'd out; host divides and adds
    the cnt2*v global/random contribution.
  * Blocks are processed in pairs: per pair, the 4 even-subhead score
    maps share one PSUM bank (PE rows 0-63) and the 4 odd ones another
    (rows 64-127), so the two row groups run concurrently and each
    bank is exp'd in a single ACT op.
  * DMA: qk chunks ride the sync HW ring, v chunks the vector ring,
    early out chunks the gpsimd SWDGE ring, late ones sync (idle once
    qk drains). Total HBM traffic ~2.6 MB/core vs 6.4 MB baseline.
"""

import numpy as np

B, S, H, D = 2, 2048, 16, 64
BS = 128          # block size
NB = S // BS      # 16 diagonal blocks
NPAIR = NB // 2   # 8 block pairs
G = 128           # num global tokens
SCALE = 1.0 / float(D) ** 0.5
EBIAS = -1.5      # global exp shift; cancels in the normalize
NCORES = 8
HPC = 4           # heads per core
QCH = [1, 1, 2, 4, 8]       # s-blocks per qk DMA chunk (small first)
QOF = [0, 1, 2, 4, 8]
VCH = [2, 2, 4, 8]          # v chunks
VOF = [0, 2, 4, 8]
OCH = [4, 4, 4, 2, 2]       # out chunks: smaller at the end for a short tail
OOF = [0, 4, 8, 12, 14]

_cached = {}


def _build_program():
    import concourse.bass as bass
    import concourse.tile as tile
    from concourse import bacc, mybir

    f32 = mybir.dt.float32
    f16 = mybir.dt.float16
    f8 = mybir.dt.float8e4
    AF = mybir.ActivationFunctionType

    nc = bacc.Bacc(
        "TRN2",
        target_bir_lowering=False,
        debug=False,
        enable_asserts=False,
        num_devices=NCORES,
    )
    # qk chunks of [p][a(q=0,k=1)][hp][s]: partition p = (h%2)*64 + d,
    # hp = h//2. Chunk-tiled (QCH) so each load is one contiguous run.
    qk = nc.dram_tensor("qk", [128 * 2 * 2 * S], f8, kind="ExternalInput").ap()
    # v chunks of [p(token-in-block)][blk][h][d|1]: ones col D is the
    # softmax denominator.
    VROW = HPC * (D + 1)
    v = nc.dram_tensor("v", [128 * VROW * NB], f8, kind="ExternalInput").ap()
    # unnormalized PV plus rowsum column, f16, chunk-tiled (OCH);
    # normalization happens on host.
    out = nc.dram_tensor("out", [128 * VROW * NB], f16, kind="ExternalOutput").ap()

    OD = D + 2  # 8-byte-aligned per-head stride in the output PSUM bank

    with tile.TileContext(nc) as tc:
        with (
            tc.tile_pool(name="qk", bufs=1) as qkpool,
            tc.tile_pool(name="vp", bufs=1) as vpool,
            tc.tile_pool(name="wp", bufs=2) as wpool,
            tc.tile_pool(name="outp", bufs=1) as opool,
            tc.tile_pool(name="stps", bufs=2, space="PSUM") as stpool,
            tc.tile_pool(name="ops", bufs=2, space="PSUM") as oppool,
        ):
            # per-partition bias operand for the exp shift
            bias_t = qkpool.tile([128, 1], f32, tag="ebias")
            nc.gpsimd.memset(bias_t[:], EBIAS)
            # prefetch all q/k chunks up-front on the sync HW ring
            block_qk = {}  # block -> (tile, in-chunk idx)
            for ci, (off, sz) in enumerate(zip(QOF, QCH)):
                ln = sz * BS
                base = 128 * 2 * 2 * off * BS
                cnt = 128 * 2 * 2 * ln
                t = qkpool.tile([128, 2, 2, ln], f8, tag=f"qk{ci}")
                nc.sync.dma_start(
                    t[:],
                    qk[base : base + cnt].rearrange(
                        "(p a h s) -> p a h s", p=128, a=2, h=2
                    ),
                )
                for i in range(sz):
                    block_qk[off + i] = (t, i)

            vmap = {}  # block -> (chunk idx, in-chunk idx)
            for ci, (off, sz) in enumerate(zip(VOF, VCH)):
                for i in range(sz):
                    vmap[off + i] = (ci, i)
            v_chunk = {}  # chunk idx -> tile

            # all v chunks up-front on the scalar HW ring: the enqueue
            # instructions run on ACT while it would otherwise idle
            # waiting for the first qk chunk
            for ci, (off, sz) in enumerate(zip(VOF, VCH)):
                v_t = vpool.tile([128, sz, HPC, D + 1], f8, tag=f"v{ci}")
                base = 128 * VROW * off
                nc.scalar.dma_start(
                    v_t[:],
                    v[base : base + 128 * VROW * sz].rearrange(
                        "(p c h d) -> p c h d", p=128, c=sz, h=HPC
                    ),
                )
                v_chunk[ci] = v_t

            omap = {}  # block -> (chunk idx, in-chunk idx)
            for ci, (off, sz) in enumerate(zip(OOF, OCH)):
                for i in range(sz):
                    omap[off + i] = (ci, i)

            state = [None] * NPAIR

            def stage_front(t):
                """QK^T + exp for block pair t (blocks 2t, 2t+1)"""
                # score banks: one per PE row group, 2 heads x 2 blocks each
                st_e = stpool.tile([128, 4 * BS], f32, tag="st_e")
                st_o = stpool.tile([128, 4 * BS], f32, tag="st_o")
                for bi in range(2):
                    qt, idx = block_qk[2 * t + bi]
                    ssl = slice(idx * BS, (idx + 1) * BS)
                    for h in range(HPC):
                        hp, sub = divmod(h, 2)
                        st = (st_e, st_o)[sub]
                        dsl = slice(sub * 64, (sub + 1) * 64)
                        col = slice((2 * bi + hp) * BS, (2 * bi + hp + 1) * BS)
                        # S^T[k,q] = K'Q (fp8)
                        nc.tensor.matmul(
                            st[:, col],
                            lhsT=qt[dsl, 1, hp, ssl],
                            rhs=qt[dsl, 0, hp, ssl],
                            start=True, stop=True,
                        )
                w_e = wpool.tile([128, 4 * BS], f8, tag="w_e")
                nc.scalar.activation(w_e[:], st_e[:], AF.Exp, scale=SCALE, bias=bias_t[:])
                w_o = wpool.tile([128, 4 * BS], f8, tag="w_o")
                nc.scalar.activation(w_o[:], st_o[:], AF.Exp, scale=SCALE, bias=bias_t[:])
                state[t] = {"w_e": w_e, "w_o": w_o}

            def stage_back(t):
                """PV + evacuate + store for block pair t"""
                stt = state[t]
                for bi in range(2):
                    sb = 2 * t + bi
                    vci, vbl = vmap[sb]
                    v_t = v_chunk[vci]
                    o_ps = oppool.tile([128, HPC, OD], f32, tag=f"o{bi}")
                    for h in range(HPC):
                        hp, sub = divmod(h, 2)
                        w_t = (stt["w_e"], stt["w_o"])[sub]
                        wcol = slice((2 * bi + hp) * BS, (2 * bi + hp + 1) * BS)
                        nc.tensor.matmul(
                            o_ps[:, h, 0 : D + 1],
                            lhsT=w_t[:, wcol],
                            rhs=v_t[:, vbl, h, :],
                            start=True, stop=True,
                        )
                    oci, oi = omap[sb]
                    osz = OCH[oci]
                    if oi == 0:
                        out_t = opool.tile(
                            [128, osz, HPC, D + 1], f16, tag=f"out{oci}"
                        )
                        stt[f"out{oci}"] = out_t
                    else:
                        # chunk tile lives in the state of the pair that
                        # held the chunk's first block (OOF are all even)
                        out_t = state[OOF[oci] // 2][f"out{oci}"]
                    # evacuate f32 PSUM -> f16 SBUF (GPSIMD can't read PSUM)
                    nc.vector.tensor_copy(out_t[:, oi], o_ps[:, :, 0 : D + 1])
                    if oi == osz - 1:
                        base = 128 * VROW * OOF[oci]
                        dma_eng = nc.sync if oci >= 3 else nc.gpsimd
                        dma_eng.dma_start(
                            out[base : base + 128 * VROW * osz].rearrange(
                                "(p c h d) -> p c h d", p=128, c=osz, h=HPC
                            ),
                            out_t[:],
                        )

            # 1-pair software skew: PE runs pair t's QK^T while ACT exps
            # pair t-1, then PE does PV(t-1).
            SKEW = 1
            for t in range(NPAIR + SKEW):
                if t < NPAIR:
                    stage_front(t)
                if t >= SKEW:
                    stage_back(t - SKEW)
    nc.compile()
    return nc


def _get_nc():
    if "nc" not in _cached:
        _cached["nc"] = _build_program()
    return _cached["nc"]


def _make_in_maps(q, k, v, rand_indices):
    import ml_dtypes

    f8 = ml_dtypes.float8_e4m3

    q = np.asarray(q, dtype=np.float32)
    k = np.asarray(k, dtype=np.float32)
    v = np.asarray(v, dtype=np.float32)

    in_maps = []
    for c in range(NCORES):
        b, hg = divmod(c, 4)
        hsl = slice(HPC * hg, HPC * (hg + 1))
        # (S, HPC, D) -> (HPC, D, S); partition p = (h%2)*64 + d, free
        # axes (a, hp, s)
        qT = q[b, :, hsl, :].transpose(1, 2, 0)  # (HPC, D, S)
        kT = k[b, :, hsl, :].transpose(1, 2, 0)
        # full[a][sub][hp][d][s] -> [p = sub*64+d][a][hp][s]
        full = np.stack([qT, kT])  # (2, HPC, D, S)
        full = full.reshape(2, 2, 2, D, S)  # (a, hp, sub, d, s)
        full = full.transpose(2, 3, 0, 1, 4)  # (sub, d, a, hp, s)
        full = full.reshape(128, 2, 2, S).astype(f8)
        qkc = np.empty(128 * 2 * 2 * S, f8)
        pos = 0
        for off, sz in zip(QOF, QCH):
            ch = np.ascontiguousarray(full[:, :, :, off * BS : (off + sz) * BS])
            qkc[pos : pos + ch.size] = ch.ravel()
            pos += ch.size

        vc = v[b, :, hsl, :]  # (S, HPC, D) f32
        vhl = np.zeros((S, HPC, D + 1), np.float32)
        vhl[:, :, 0:D] = vc
        vhl[:, :, D] = 1.0  # softmax denominator column
        vhl = vhl.reshape(NB, 128, HPC, D + 1).astype(f8)
        vflat = np.empty(128 * HPC * (D + 1) * NB, f8)
        pos = 0
        for off, sz in zip(VOF, VCH):
            ch = np.ascontiguousarray(vhl[off : off + sz].transpose(1, 0, 2, 3))
            vflat[pos : pos + ch.size] = ch.ravel()
            pos += ch.size
        in_maps.append({"qk": qkc, "v": vflat})
    return in_maps


def _unpack_out(o):
    """OCH-chunk-tiled flat f16 -> (S, HPC, D+1) f32"""
    res = np.empty((NB, 128, HPC, D + 1), np.float32)
    o = np.asarray(o, dtype=np.float32)
    pos = 0
    for off, sz in zip(OOF, OCH):
        n = 128 * sz * HPC * (D + 1)
        ch = o[pos : pos + n].reshape(128, sz, HPC, D + 1)
        res[off : off + sz] = ch.transpose(1, 0, 2, 3)
        pos += n
    return res.reshape(S, HPC, D + 1)


def _assemble(results, v, rand_indices):
    out = np.empty((B, S, H, D), dtype=np.float32)
    for c in range(NCORES):
        b, hg = divmod(c, 4)
        o = _unpack_out(results[c]["out"])  # (S, HPC, D+1): [o_unnorm | rowsum]
        out[b, :, HPC * hg : HPC * (hg + 1), :] = o[:, :, 0:D] / o[:, :, D : D + 1]
    # global + random contributions: out[:, s] += cnt2[s] * v[:, s]
    ri = np.asarray(rand_indices).astype(np.int64).ravel()
    cnt = np.bincount(ri, minlength=S).astype(np.float32)
    cnt[:G] += 1.0
    nz = np.nonzero(cnt)[0]
    out[:, nz] += cnt[nz, None, None] * np.asarray(v, np.float32)[:, nz]
    return out


def _run(q, k, v, attn_mask, rand_indices, trace=False, trace_kwargs=None):
    from concourse.bass_utils import run_bass_kernel_spmd

    nc = _get_nc()
    in_maps = _make_in_maps(q, k, v, rand_indices)
    res = run_bass_kernel_spmd(
        nc,
        in_maps,
        list(range(NCORES)),
        trace=trace,
        **(trace_kwargs or {}),
    )
    return _assemble(res.results, v, rand_indices), res


def _reference_fallback(q, k, v, attn_mask, rand_indices):
    """Numpy replica of the reference for the (never expected per spec)
    case of a non-zero attn_mask."""
    q = np.asarray(q, np.float32)
    k = np.asarray(k, np.float32)
    v = np.asarray(v, np.float32)
    m = np.asarray(attn_mask, np.float32)
    ri = np.asarray(rand_indices).astype(np.int64).ravel()

    def softmax(x):
        x = x - x.max(axis=-1, keepdims=True)
        e = np.exp(x)
        return e / e.sum(axis=-1, keepdims=True)

    qb = q.reshape(B, NB, BS, H, D)
    kb = k.reshape(B, NB, BS, H, D)
    vb = v.reshape(B, NB, BS, H, D)
    scores = np.einsum("bnqhd,bnkhd->bnhqk", qb, kb) * SCALE
    mb = m.reshape(B, H, NB, BS, NB, BS)
    idx = np.arange(NB)
    diag = mb[:, :, idx, :, idx, :]  # (NB,B,H,BS,BS)
    scores = scores + diag.transpose(1, 0, 2, 3, 4)
    w = softmax(scores)
    out = np.einsum("bnhqk,bnkhd->bnqhd", w, vb).reshape(B, S, H, D)

    gq = q[:, :G]
    gv = v[:, :G]
    gs = np.einsum("bghd,bshd->bhgs", gq, k) * SCALE + m[:, :, :G, :]
    gw = softmax(gs)
    out[:, :G] += gv * gw.sum(axis=-1).transpose(0, 2, 1)[..., None]

    rq = q[:, ri]
    rv = v[:, ri]
    rs = np.einsum("brhd,bshd->bhrs", rq, k) * SCALE + m[:, :, ri, :]
    rw = softmax(rs)
    rowsum = rw.sum(axis=-1).transpose(0, 2, 1)  # (B,R,H)
    contrib = rv * rowsum[..., None]
    np.add.at(out, (slice(None), ri), contrib)
    return out


def kernel(q, k, v, attn_mask, rand_indices):
    am = np.asarray(attn_mask)
    if am.any():
        return _reference_fallback(q, k, v, attn_mask, rand_indices)
    out, _ = _run(q, k, v, attn_mask, rand_indices, trace=False)
    return out
